# revision 31
# baseline (speedup 1.0000x reference)
"""Trainium2 Bass kernel for nn_Attention_60833916781258 (GAT-style complex attention).

Reference computation (B=2, N=4096, F=128, U=64):
    X_re = H_re @ W ; X_im = H_im @ W
    s = X @ a_1 ; n = X @ a_2 (per re/im)
    E = leaky_relu(s_i + n_j, 0.2)
    alpha1 = softmax(E_re + NEG_BIG*(1-A)) ; alpha2 = softmax(E_im)
    out_re = alpha1 @ X_re - alpha2 @ X_im ; out_im = alpha1 @ X_im + alpha2 @ X_re

Sharding: 8 cores; core c handles batch b=c//4, query-row block rb=c%4 (1024 rows).

Math trick: exp(lrelu(t)) = e^{0.2t} * max(e^{0.8t}, 1).  The per-row factor
e^{0.2 s_i} cancels in the row softmax, so the unnormalized weights are
    u[j,i] = A[i,j] * max(e^{0.8 s_i + n_j}, e^{0.2 n_j}) / 16
(1/16 keeps fp16 row-sum accumulators below 65504; it cancels too).
Per 128-key chunk: ONE scalar-engine Exp (bias = n_j - ln16 per partition),
ONE 4x-mode DVE tensor_scalar max (floor = e^{0.2 n_j}/16 per partition) and,
for E1 only, ONE 2x-mode DVE fp16 multiply by the 0/1 adjacency tile.

Layout is key-major [j, i] so the alpha@X contraction runs on the tensor
engine without transposing attention tiles.  Row sums: u accumulated in fp16
on DVE, v summed via interleaved ones-matmuls on the PE.

X/s/n setup: per key chunk ONE 66-col matmul (lhsT = ht chunk as weights,
rhs = [W | W@a1 | W@a2]) gives X, s and n in key-major layout directly; s is
broadcast separately via a rank-1 matmul.  The u and v matmuls share the
xcat weights (single ldweights per chunk).  The epilogue transposes the
numerators on the PE in fp16 (4 row blocks per PSUM bank), combines straight
from PSUM, and writes one [1024, 128] fp32 output (re | im) per core.

Note: dma_start_transpose (XBAR) looked attractive for the transposes, but
InstDmaTransposeAnt is invisible to the tile framework's dependency tracking
(get_accessed_tiles returns []), so it races with producers/consumers and
intermittently corrupts results.  Keep transposes on the PE.
"""

import sys

if "/opt/trn_rl_repo" not in sys.path:
    sys.path.insert(0, "/opt/trn_rl_repo")

import math

import numpy as np

import concourse.bass as bass
import concourse.tile as tile
from concourse import bacc, mybir
from concourse.bass_utils import run_bass_kernel_spmd

B, N, F, U = 2, 4096, 128, 64
NCORES = 8
ROWS = N * B // NCORES  # 1024 query rows per core
NCHUNK = N // 128  # 32 key chunks of 128
LN16 = math.log(16.0)
A_ = mybir.AluOpType
AF = mybir.ActivationFunctionType
f32 = mybir.dt.float32
f16 = mybir.dt.float16

_PROGRAM_CACHE = {}


def _build_program():
    if "nc" in _PROGRAM_CACHE:
        return _PROGRAM_CACHE["nc"]

    nc = bacc.Bacc("TRN2", target_bir_lowering=False, debug=False, num_devices=NCORES)
    from concourse.tile_rust import add_dep_helper

    _pe_prev = [None]

    def mm(out, lhsT, rhs, reuse=False, **kw):
        bi = nc.tensor.matmul(out, lhsT=lhsT, rhs=rhs, **kw)
        if reuse:
            bi.ins.ldweights = False
        if _pe_prev[0] is not None:
            add_dep_helper(bi.ins, _pe_prev[0], sync=False, reason="pe order")
        _pe_prev[0] = bi.ins
        return bi

    dp = nc.dram_tensor
    ht_in = {
        "re": dp("ht_re", [F, N], f16, kind="ExternalInput").ap(),
        "im": dp("ht_im", [F, N], f16, kind="ExternalInput").ap(),
    }
    wsn_in = dp("wsn", [F, 66], f16, kind="ExternalInput").ap()
    wa8_in = dp("wa8", [F, 1], f32, kind="ExternalInput").ap()
    amul_in = dp("amul", [N, ROWS], f16, kind="ExternalInput").ap()
    ident_in = dp("ident", [128, 128], f32, kind="ExternalInput").ap()
    identh_in = dp("identh", [128, 128], f16, kind="ExternalInput").ap()
    o_cat = dp("ocat", [ROWS, 128], f32, kind="ExternalOutput").ap()

    with tile.TileContext(nc) as tc:
        with tc.tile_pool(name="cst", bufs=1) as cst:
            # ---- constants (wsn = [W | W@a1 | W@a2], wa8 = 0.8*W@a1, host-side)
            wsn_sb = cst.tile([F, 66], f16, tag="wsn", name="wsn")
            nc.sync.dma_start(wsn_sb[:], wsn_in[:])
            wa8_sb = cst.tile([F, 1], f32, tag="wa8", name="wa8")
            nc.sync.dma_start(wa8_sb[:], wa8_in[:])
            ident_sb = cst.tile([128, 128], f32, tag="ident", name="ident")
            nc.scalar.dma_start(ident_sb[:], ident_in[:])
            identh_sb = cst.tile([128, 128], f16, tag="identh", name="identh")
            nc.scalar.dma_start(identh_sb[:], identh_in[:])
            ones16_sb = cst.tile([128, 1], f16, tag="ones16", name="ones16")
            nc.gpsimd.memset(ones16_sb[:], 1.0)
            ones128_sb = cst.tile([128, 128], f16, tag="ones128", name="ones128")
            nc.gpsimd.memset(ones128_sb[:], 1.0)
            negln16_sb = cst.tile([128, 1], f32, tag="negln16", name="negln16")
            nc.gpsimd.memset(negln16_sb[:], -LN16)

            # ---- ht in 4 col-pieces per tensor, split across both HWDGE queues
            ht_sb = {}
            for qi, nm in enumerate(("re", "im")):
                t = cst.tile([F, N], f16, tag=f"ht_{nm}", name=f"ht_{nm}")
                ht_sb[nm] = t
                eng = nc.gpsimd if nm == "re" else nc.scalar
                for g in range(4):
                    eng.dma_start(
                        t[:, 1024 * g : 1024 * (g + 1)],
                        ht_in[nm][:, 1024 * g : 1024 * (g + 1)],
                    )

            xsn_sb = {}  # key-major [key, 0:64=X | 64=s | 65=n] per chunk
            xsn3 = {}
            for nm in ("re", "im"):
                t = cst.tile([128, NCHUNK * 66], f16, tag=f"xsn_{nm}", name=f"xsn_{nm}")
                xsn_sb[nm] = t
                xsn3[nm] = t[:].rearrange("p (c u) -> p c u", u=66)
            xcat_sb = cst.tile([128, NCHUNK * 128], f16, tag="xcat", name="xcat")
            xcat3 = xcat_sb[:].rearrange("p (c u) -> p c u", u=128)

            sbc8_sb = {
                nm: cst.tile([128, ROWS], f16, tag=f"sbc8_{nm}", name=f"sbc8_{nm}")
                for nm in ("re", "im")
            }
            bias_re = cst.tile([128, NCHUNK], f32, tag="bias_re", name="bias_re")
            bias_im = cst.tile([128, NCHUNK], f32, tag="bias_im", name="bias_im")
            floor_re = cst.tile([128, NCHUNK], f32, tag="floor_re", name="floor_re")
            floor_im = cst.tile([128, NCHUNK], f32, tag="floor_im", name="floor_im")
            accu_sb = cst.tile([128, ROWS], f16, tag="accu", name="accu")
            nc.gpsimd.memset(accu_sb[:], 0.0)

            # ---- setup matmuls: sbc8 (rank-1 s broadcast) and XT66 per tensor
            with (
                tc.tile_pool(name="psB", bufs=2, space="PSUM") as psB,
                tc.tile_pool(name="psX", bufs=3, space="PSUM") as psX,
            ):
                # w18[f,p] = 0.8*(W@a1)[f] for all p
                w18_sb = cst.tile([128, 128], f16, tag="w18", name="w18")
                nc.vector.tensor_scalar(
                    w18_sb[:],
                    ones128_sb[:],
                    wa8_sb[:, 0:1],
                    None,
                    op0=A_.mult,
                )
                for nm in ("re", "im"):
                    # sbc8: s for own rows (key cols 0:1024), broadcast to all
                    # partitions, scaled by 0.8
                    sb_ps = psB.tile([128, 512], f32, tag="sb_ps", name="sb_ps")
                    for h in range(2):
                        mm(
                            sb_ps[:],
                            w18_sb[:],
                            ht_sb[nm][:, 512 * h : 512 * (h + 1)],
                            start=True,
                            stop=True,
                        )
                        if h == 0:
                            nc.scalar.copy(sbc8_sb[nm][:, 0:512], sb_ps[:])
                        else:
                            nc.vector.tensor_copy(sbc8_sb[nm][:, 512:1024], sb_ps[:])
                # X/s/n: per key chunk, ONE 66-col matmul with lhsT = ht chunk
                # (weights) and rhs = wsn; 7 chunks batched per PSUM bank.
                for nm in ("re", "im"):
                    for t0 in range(0, NCHUNK, 7):
                        tn = min(7, NCHUNK - t0)
                        xg = psX.tile([128, 462], f32, tag="xg", name="xg")
                        xg3 = xg[:].rearrange("p (c u) -> p c u", u=66)
                        for m in range(tn):
                            k = t0 + m
                            mm(
                                xg3[:, m, :],
                                ht_sb[nm][:, 128 * k : 128 * (k + 1)],
                                wsn_sb[:],
                                start=True,
                                stop=True,
                            )
                        nc.scalar.copy(
                            xsn3[nm][:, t0 : t0 + tn, 0:64], xg3[:, 0:tn, 0:64]
                        )
                        nc.vector.tensor_copy(
                            xsn3[nm][:, t0 : t0 + tn, 64:66], xg3[:, 0:tn, 64:66]
                        )

            # assemble matmul weights: xcat = [X_re | X_im]
            nc.vector.tensor_copy(xcat3[:, :, 0:64], xsn3["re"][:, :, 0:64])
            nc.vector.tensor_copy(xcat3[:, :, 64:128], xsn3["im"][:, :, 0:64])

            # scalar preps: bias = n - ln16 ; floor = e^{0.2n}/16  (n at col 65)
            n_ap = {nm: xsn3[nm][:, :, 65:66] for nm in ("re", "im")}
            b3 = lambda t: t[:].rearrange("p (k o) -> p k o", o=1)
            nc.vector.tensor_scalar_add(b3(bias_re), n_ap["re"], -LN16)
            nc.vector.tensor_scalar_add(b3(bias_im), n_ap["im"], -LN16)
            nc.scalar.activation(
                b3(floor_re), n_ap["re"], AF.Exp, bias=negln16_sb[:], scale=0.2
            )
            nc.scalar.activation(
                b3(floor_im), n_ap["im"], AF.Exp, bias=negln16_sb[:], scale=0.2
            )

            # ---- main loop over 32 key chunks
            with (
                tc.tile_pool(name="psM", bufs=1, space="PSUM") as psM,
                tc.tile_pool(name="amp", bufs=10) as am_pool,
                tc.tile_pool(name="ep_", bufs=4) as e_pool,
                tc.tile_pool(name="uvp", bufs=6) as uv_pool,
                tc.tile_pool(name="fin", bufs=1) as fin,
            ):
                psum_u = [
                    psM.tile([128, 512], f32, tag=f"pu{h}", name=f"pu{h}")
                    for h in range(2)
                ]
                psum_v = [
                    psM.tile([128, 512], f32, tag=f"pv{h}", name=f"pv{h}")
                    for h in range(2)
                ]
                psum_sv = psM.tile([128, 512], f32, tag="psv", name="psv")

                first_sv = [True]
                for k in range(NCHUNK):
                    am_t = am_pool.tile([128, ROWS], f16, tag="am", name="am")
                    nc.sync.dma_start(am_t[:], amul_in[128 * k : 128 * (k + 1), :])
                    e1 = e_pool.tile([128, ROWS], f16, tag="e1", name="e1")
                    nc.scalar.activation(
                        e1[:], sbc8_sb["re"][:], AF.Exp, bias=bias_re[:, k : k + 1]
                    )
                    m1 = uv_pool.tile([128, ROWS], f16, tag="m1", name="m1")
                    nc.vector.tensor_scalar_max(m1[:], e1[:], floor_re[:, k : k + 1])
                    u_t = uv_pool.tile([128, ROWS], f16, tag="u", name="u")
                    nc.vector.tensor_tensor(u_t[:], m1[:], am_t[:], op=A_.mult)
                    e2 = e_pool.tile([128, ROWS], f16, tag="e2", name="e2")
                    nc.scalar.activation(
                        e2[:], sbc8_sb["im"][:], AF.Exp, bias=bias_im[:, k : k + 1]
                    )
                    v_t = uv_pool.tile([128, ROWS], f16, tag="v", name="v")
                    nc.vector.tensor_scalar_max(v_t[:], e2[:], floor_im[:, k : k + 1])
                    nc.vector.tensor_tensor(accu_sb[:], accu_sb[:], u_t[:], op=A_.add)

                    st, sp = (k == 0), (k == NCHUNK - 1)
                    for h in range(2):
                        mm(
                            psum_u[h][:],
                            xcat3[:, k, :],
                            u_t[:, 512 * h : 512 * (h + 1)],
                            start=st,
                            stop=sp,
                            reuse=(h != 0),
                        )
                    for h in range(2):
                        mm(
                            psum_v[h][:],
                            xcat3[:, k, :],
                            v_t[:, 512 * h : 512 * (h + 1)],
                            start=st,
                            stop=sp,
                            reuse=True,
                        )
                    for h in range(2):
                        mm(
                            psum_sv[32 * h : 32 * h + 1, :],
                            ones16_sb[:],
                            v_t[:, 512 * h : 512 * (h + 1)],
                            start=st,
                            stop=sp,
                            reuse=not first_sv[0],
                        )
                        first_sv[0] = False

                # drains: numerators to fp16 SBUF, row sums to suv
                cu_sb = fin.tile([128, ROWS], f16, tag="cu", name="cu")
                cv_sb = fin.tile([128, ROWS], f16, tag="cv", name="cv")
                for h in range(2):
                    nc.scalar.copy(cu_sb[:, 512 * h : 512 * (h + 1)], psum_u[h][:])
                    nc.vector.tensor_copy(
                        cv_sb[:, 512 * h : 512 * (h + 1)], psum_v[h][:]
                    )
                # su/sv rows at partitions 0 and 32 (PE base-partition rule)
                su_sb = fin.tile([128, 512], f32, tag="su", name="su")
                sv_sb = fin.tile([128, 512], f32, tag="sv", name="sv")
                nc.vector.tensor_copy(sv_sb[:], psum_sv[:])
                with tc.tile_pool(name="psU", bufs=1, space="PSUM") as psU:
                    su_ps = psU.tile([128, 512], f32, tag="su_ps", name="su_ps")
                    for h in range(2):
                        mm(
                            su_ps[32 * h : 32 * h + 1, :],
                            ones16_sb[:],
                            accu_sb[:, 512 * h : 512 * (h + 1)],
                            start=True,
                            stop=True,
                            reuse=(h != 0),
                        )
                    nc.scalar.copy(su_sb[:], su_ps[:])

            # ---- epilogue
            with (
                tc.tile_pool(name="psE", bufs=1, space="PSUM") as psE,
                tc.tile_pool(name="ep2", bufs=1) as ep2,
            ):
                # pipelined per row-block: transposes (su, sv, tu, tv) ->
                # per-block reciprocal -> combine -> per-block output DMA,
                # so block 0's chain completes while block 7 still transposes
                rsT_ps = psE.tile([128, 16], f32, tag="rsT", name="rsT")
                tu_ps = [
                    psE.tile([128, 512], f16, tag=f"tu{b}", name=f"tu{b}")
                    for b in range(2)
                ]
                tv_ps = [
                    psE.tile([128, 512], f16, tag=f"tv{b}", name=f"tv{b}")
                    for b in range(2)
                ]
                ocat_sb = ep2.tile([128, 8 * 128], f32, tag="ocat", name="ocat")
                ocat3 = ocat_sb[:].rearrange("p (c u) -> p c u", u=128)
                oc_view = o_cat[:].rearrange("(c p) u -> p c u", p=128)
                rr_sb = ep2.tile([128, 16], f32, tag="rr", name="rr")
                for it in range(8):
                    h, j = divmod(it, 4)
                    b = h
                    sl = slice(128 * it, 128 * (it + 1))
                    dl = slice(128 * j, 128 * (j + 1))
                    for si, src in enumerate((su_sb, sv_sb)):
                        mm(
                            rsT_ps[:, 2 * it + si : 2 * it + si + 1],
                            src[32 * h : 32 * h + 1, 128 * j : 128 * (j + 1)],
                            ident_sb[32 * h : 32 * h + 1, 32 * h : 32 * h + 1],
                            is_transpose=True,
                            start=True,
                            stop=True,
                        )
                    mm(
                        tu_ps[b][:, dl],
                        cu_sb[:, sl],
                        identh_sb[:],
                        is_transpose=True,
                        start=True,
                        stop=True,
                    )
                    mm(
                        tv_ps[b][:, dl],
                        cv_sb[:, sl],
                        identh_sb[:],
                        is_transpose=True,
                        start=True,
                        stop=True,
                    )
                    nc.vector.reciprocal(
                        rr_sb[:, 2 * it : 2 * it + 2], rsT_ps[:, 2 * it : 2 * it + 2]
                    )
                    ru = rr_sb[:, 2 * it : 2 * it + 1]
                    rv = rr_sb[:, 2 * it + 1 : 2 * it + 2]
                    tub = tu_ps[b][:, dl]
                    tvb = tv_ps[b][:, dl]
                    # out_re = tu_re*ru - tv_im*rv ; out_im = tu_im*ru + tv_re*rv
                    wv = ep2.tile([128, 128], f32, tag=f"wv{it}", name=f"wv{it}")
                    nc.scalar.activation(
                        wv[:, 0:64], tvb[:, 64:128], AF.Copy, scale=rv
                    )
                    nc.scalar.activation(
                        wv[:, 64:128], tvb[:, 0:64], AF.Copy, scale=rv
                    )
                    nc.vector.scalar_tensor_tensor(
                        ocat3[:, it, 0:64],
                        tub[:, 0:64],
                        ru,
                        wv[:, 0:64],
                        A_.mult,
                        A_.subtract,
                    )
                    nc.vector.scalar_tensor_tensor(
                        ocat3[:, it, 64:128],
                        tub[:, 64:128],
                        ru,
                        wv[:, 64:128],
                        A_.mult,
                        A_.add,
                    )
                    nc.sync.dma_start(
                        oc_view[:, it : it + 1, :], ocat3[:, it : it + 1, :]
                    )

    nc.compile()
    _PROGRAM_CACHE["nc"] = nc
    return nc


# ---------------------------------------------------------------- host wrapper


def _make_in_maps(H_re, H_im, A, W, a_1, a_2):
    W32 = np.asarray(W, np.float32)
    acat = np.concatenate(
        [np.asarray(a_1, np.float32), np.asarray(a_2, np.float32)], axis=1
    )
    wa = W32 @ acat  # [F, 2] = [W@a1 | W@a2]
    wsn = np.concatenate([W32, wa], axis=1).astype(np.float16)  # [F, 66]
    wa8 = (0.8 * wa[:, 0:1]).astype(np.float32)
    shared = {
        "wsn": wsn,
        "wa8": wa8,
        "ident": np.eye(128, dtype=np.float32),
        "identh": np.eye(128, dtype=np.float16),
    }
    in_maps = []
    for c in range(NCORES):
        b, rb = divmod(c, NCORES // B)
        r0 = rb * ROWS
        hre = np.asarray(H_re[b], np.float32)
        him = np.asarray(H_im[b], np.float32)
        ab = np.asarray(A[b], np.float32)
        # key order rolled so this core's own query rows come first
        amul = np.ascontiguousarray(
            np.roll(ab[r0 : r0 + ROWS].T, -r0, axis=0).astype(np.float16)
        )
        in_maps.append(
            {
                **shared,
                "ht_re": np.ascontiguousarray(
                    np.roll(hre, -r0, axis=0).T.astype(np.float16)
                ),
                "ht_im": np.ascontiguousarray(
                    np.roll(him, -r0, axis=0).T.astype(np.float16)
                ),
                "amul": amul,
            }
        )
    return in_maps


def kernel(H_re, H_im, A, W, a_1, a_2):
    nc = _build_program()
    in_maps = _make_in_maps(H_re, H_im, A, W, a_1, a_2)
    res = run_bass_kernel_spmd(nc, in_maps, list(range(NCORES)))
    out_re = np.empty((B, N, U), np.float32)
    out_im = np.empty((B, N, U), np.float32)
    for c in range(NCORES):
        b, rb = divmod(c, NCORES // B)
        r0 = rb * ROWS
        oc = res.results[c]["ocat"]
        out_re[b, r0 : r0 + ROWS] = oc[:, 0:64]
        out_im[b, r0 : r0 + ROWS] = oc[:, 64:128]
    return out_re, out_im
